# revision 1
# baseline (speedup 1.0000x reference)
"""Trainium2 Bass kernel for BatchAllTripletWithClustersLossSemiHard.

Strategy (data-parallel over anchors, 8 cores):
  For anchor i only same-label j matter (~B/NCLASS of them). Build compact
  (i,j) pair list per core; each pair is one SBUF partition row over k=0..B-1:
      W[p,k] = relu( w_j*(1 + d_ij - d_ik) + madd[p,k] )
  where madd is -BIG*w on masked-out k (label-rank semi-hard logic, k==i,
  k==j, padding) and 0 elsewhere -- all label logic precomputed on host.
  d_ij - d_ik = V[i,j] - V[i,k] with V[i,k] = 2*x_i.x_k - |x_k|^2 (the |x_i|^2
  terms cancel), V computed on device with PE matmuls.  One ScalarE activation
  per row-chunk produces W and its row-sum; one VectorE tensor_scalar produces
  the positive-count row-sum.  Partials are summed on-chip, the 8 per-core
  (sum, count) pairs are combined on host:  loss = S / (C + eps).
"""

import numpy as np

import concourse.bass as bass
import concourse.tile as tile
from concourse import bacc, mybir
from concourse.bass import IndirectOffsetOnAxis
from concourse.bass_utils import run_bass_kernel_spmd

EPS = 1e-8
BIG = 1e9
B, D, NCORES = 384, 512, 8
PERCORE = B // NCORES  # 48
P = 128
DT = mybir.dt.float32


def _host_prep(labels, clusters, weights):
    """Per-core pair tables (pure label logic, no embedding data)."""
    labels = np.asarray(labels).astype(np.int64)
    clusters = np.asarray(clusters).astype(np.int64)
    weights = np.asarray(weights).astype(np.float32)

    leq = labels[None, :] == labels[:, None]
    rank = np.cumsum(leq.astype(np.int64), axis=1) - 1
    first = leq & (rank % 2 == 1)
    second = leq & (rank % 2 == 0)
    pbase = ~first   # k-mask for in-cluster (i,j) pairs
    qbase = ~second  # k-mask for out-of-cluster pairs

    all_pairs = []
    for c in range(NCORES):
        pairs = []
        for i in range(c * PERCORE, (c + 1) * PERCORE):
            for j in np.where(leq[i])[0]:
                if j != i:
                    pairs.append((i, int(j)))
        all_pairs.append(pairs)
    NP = ((max(len(p) for p in all_pairs) + P - 1) // P) * P

    tables = []
    for c in range(NCORES):
        pairs = all_pairs[c]
        gidx = np.zeros((NP, 1), np.int32)
        fidx = np.zeros((NP, 1), np.int32)
        wvec = np.ones((NP, 1), np.float32)
        madd = np.full((NP, B), -BIG, np.float32)
        for p, (i, j) in enumerate(pairs):
            il = i - c * PERCORE
            gidx[p, 0] = il
            fidx[p, 0] = il * B + j
            w = float(weights[labels[j]])
            wvec[p, 0] = w
            base = pbase[i] if clusters[i] == clusters[j] else qbase[i]
            mask = base.copy()
            mask[i] = False
            mask[j] = False
            madd[p, :] = np.where(mask, 0.0, np.float32(-BIG * w))
        tables.append(dict(gidx=gidx, fidx=fidx, wvec=wvec, madd=madd))
    return tables, NP


def _build_program(NP):
    NCH = NP // P          # pair-row chunks
    NDC = D // P           # contraction chunks (4)

    nc = bacc.Bacc("TRN2", target_bir_lowering=False, debug=False,
                   num_devices=NCORES)

    xt = nc.dram_tensor("xt", [D, B], DT, kind="ExternalInput")
    xmyT2 = nc.dram_tensor("xmyT2", [D, PERCORE], DT, kind="ExternalInput")
    madd = nc.dram_tensor("madd", [NP, B], DT, kind="ExternalInput")
    gidx = nc.dram_tensor("gidx", [NP, 1], mybir.dt.int32, kind="ExternalInput")
    fidx = nc.dram_tensor("fidx", [NP, 1], mybir.dt.int32, kind="ExternalInput")
    wvec = nc.dram_tensor("wvec", [NP, 1], DT, kind="ExternalInput")
    out = nc.dram_tensor("out", [1, 2], DT, kind="ExternalOutput")

    with tile.TileContext(nc) as tc:
        with (
            tc.tile_pool(name="cst", bufs=1) as cst,
            tc.tile_pool(name="xtp", bufs=NDC) as xtp,
            tc.tile_pool(name="sq", bufs=2) as sqp,
            tc.tile_pool(name="vg", bufs=3) as vgp,
            tc.tile_pool(name="wrk", bufs=3) as wrk,
            tc.tile_pool(name="sm", bufs=4) as sm,
            tc.tile_pool(name="ps", bufs=2, space="PSUM") as ps,
            tc.tile_pool(name="dram", bufs=1, space="DRAM") as dram,
        ):
            # --- load inputs ---
            xt_t = []
            for dc in range(NDC):
                t = xtp.tile([P, B], DT, tag="xt")
                nc.sync.dma_start(t[:], xt[dc * P:(dc + 1) * P, :])
                xt_t.append(t)
            xmy_t = []
            for dc in range(NDC):
                t = xtp.tile([P, PERCORE], DT, tag="xmy")
                nc.sync.dma_start(t[:], xmyT2[dc * P:(dc + 1) * P, :])
                xmy_t.append(t)
            gidx_t = cst.tile([P, NCH], mybir.dt.int32)
            fidx_t = cst.tile([P, NCH], mybir.dt.int32)
            wv_t = cst.tile([P, NCH], DT)
            for c in range(NCH):
                sl = slice(c * P, (c + 1) * P)
                nc.sync.dma_start(gidx_t[:, c:c + 1], gidx[sl, :])
                nc.sync.dma_start(fidx_t[:, c:c + 1], fidx[sl, :])
                nc.sync.dma_start(wv_t[:, c:c + 1], wvec[sl, :])

            negones = cst.tile([P, PERCORE], DT)
            nc.vector.memset(negones[:], -1.0)
            ones1 = cst.tile([P, 1], DT)
            nc.vector.memset(ones1[:], 1.0)

            # --- V[i,k] = 2 x_i.x_k - |x_k|^2  (PE) ---
            v_psum = ps.tile([PERCORE, B], DT)
            for dc in range(NDC):
                nc.tensor.matmul(v_psum[:], lhsT=xmy_t[dc][:], rhs=xt_t[dc][:],
                                 start=(dc == 0), stop=False)
            for dc in range(NDC):
                xsq = sqp.tile([P, B], DT, tag="xsq")
                nc.vector.tensor_mul(xsq[:], xt_t[dc][:], xt_t[dc][:])
                nc.tensor.matmul(v_psum[:], lhsT=negones[:], rhs=xsq[:],
                                 start=False, stop=(dc == NDC - 1))

            v_sb = wrk.tile([PERCORE, B], DT, tag="vsb")
            nc.scalar.copy(v_sb[:], v_psum[:])
            v_flat = dram.tile([PERCORE * B, 1], DT)
            v_rows = v_flat[:].rearrange("(a b) o -> a (b o)", b=B)
            nc.sync.dma_start(v_rows, v_sb[:])

            # --- per-pair-chunk pipeline ---
            sacc = cst.tile([P, NCH], DT)
            cacc = cst.tile([P, NCH], DT)
            for c in range(NCH):
                vg = vgp.tile([P, B], DT, tag="vg")
                nc.gpsimd.indirect_dma_start(
                    out=vg[:], out_offset=None, in_=v_rows,
                    in_offset=IndirectOffsetOnAxis(ap=gidx_t[:, c:c + 1], axis=0))
                g = sm.tile([P, 1], DT, tag="g")
                nc.gpsimd.indirect_dma_start(
                    out=g[:], out_offset=None, in_=v_flat[:],
                    in_offset=IndirectOffsetOnAxis(ap=fidx_t[:, c:c + 1], axis=0))

                mt = vgp.tile([P, B], DT, tag="mt")
                nc.sync.dma_start(mt[:], madd[c * P:(c + 1) * P, :])

                wcol = wv_t[:, c:c + 1]
                # cvec = w*(1-g)
                cvec = sm.tile([P, 1], DT, tag="cvec")
                nc.vector.tensor_scalar(cvec[:], g[:], -1.0, 1.0,
                                        op0=mybir.AluOpType.mult,
                                        op1=mybir.AluOpType.add)
                nc.vector.tensor_scalar(cvec[:], cvec[:], wcol, None,
                                        op0=mybir.AluOpType.mult)
                # t = w*Vg + madd_w
                t = vgp.tile([P, B], DT, tag="t")
                nc.vector.scalar_tensor_tensor(
                    t[:], in0=vg[:], scalar=wcol, in1=mt[:],
                    op0=mybir.AluOpType.mult, op1=mybir.AluOpType.add)
                # W = relu(t + cvec), row-sum into sacc[:, c]
                w_tile = vgp.tile([P, B], DT, tag="w")
                nc.scalar.activation(w_tile[:], t[:],
                                     mybir.ActivationFunctionType.Relu,
                                     bias=cvec[:, 0:1], scale=1.0,
                                     accum_out=sacc[:, c:c + 1])
                # count = sum(W > EPS) into cacc[:, c]
                cl = vgp.tile([P, B], DT, tag="cl")
                nc.vector.tensor_scalar(cl[:], w_tile[:], EPS, None,
                                        op0=mybir.AluOpType.is_gt,
                                        op1=mybir.AluOpType.add,
                                        accum_out=cacc[:, c:c + 1])

            # --- final reduction ---
            red = cst.tile([P, 2], DT)
            nc.vector.tensor_reduce(red[:, 0:1], sacc[:], mybir.AxisListType.X,
                                    mybir.AluOpType.add)
            nc.vector.tensor_reduce(red[:, 1:2], cacc[:], mybir.AxisListType.X,
                                    mybir.AluOpType.add)
            f_psum = ps.tile([1, 2], DT)
            nc.tensor.matmul(f_psum[:], lhsT=ones1[:], rhs=red[:],
                             start=True, stop=True)
            out_sb = cst.tile([1, 2], DT)
            nc.scalar.copy(out_sb[:], f_psum[:])
            nc.sync.dma_start(out[:, :], out_sb[:])

    nc.compile()
    return nc


def _make_in_maps(embeddings, tables):
    x = np.ascontiguousarray(np.asarray(embeddings, dtype=np.float32))
    xt = np.ascontiguousarray(x.T)  # [D, B]
    in_maps = []
    for c in range(NCORES):
        xmy = x[c * PERCORE:(c + 1) * PERCORE]  # [48, D]
        in_maps.append({
            "xt": xt,
            "xmyT2": np.ascontiguousarray(2.0 * xmy.T),
            "madd": tables[c]["madd"],
            "gidx": tables[c]["gidx"],
            "fidx": tables[c]["fidx"],
            "wvec": tables[c]["wvec"],
        })
    return in_maps


def run(embeddings, labels, clusters, weights, trace=False):
    tables, NP = _host_prep(labels, clusters, weights)
    nc = _build_program(NP)
    in_maps = _make_in_maps(embeddings, tables)
    res = run_bass_kernel_spmd(nc, in_maps, core_ids=list(range(NCORES)),
                               trace=trace)
    S = 0.0
    C = 0.0
    for r in res.results:
        S += float(r["out"][0, 0])
        C += float(r["out"][0, 1])
    loss = np.float32(np.float32(S) / np.float32(C + EPS))
    return np.asarray(loss, dtype=np.float32), res


def kernel(embeddings, labels, clusters, weights):
    loss, _ = run(embeddings, labels, clusters, weights)
    return loss


# revision 2
# speedup vs baseline: 1.5420x; 1.5420x over previous
"""Trainium2 Bass kernel for BatchAllTripletWithClustersLossSemiHard.

Strategy (data-parallel over anchors, 8 cores):
  For anchor i only same-label j matter (~B/NCLASS of them). Build compact
  (i,j) pair list per core; each pair is one SBUF partition row over k=0..B-1:
      W[p,k] = relu( w_j*(1 + d_ij - d_ik) + madd[p,k] )
  where madd is -BIG*w on masked-out k (label-rank semi-hard logic, k==i,
  k==j, padding) and 0 elsewhere -- all label logic precomputed on host.
  d_ij - d_ik = V[i,j] - V[i,k] with V[i,k] = 2*x_i.x_k - |x_k|^2 (the |x_i|^2
  terms cancel), V computed on device with PE matmuls.  Pair rows are
  broadcast from V with a one-hot selector matmul on the PE; d_ij is
  extracted with a fused one-hot dot product on the DVE.  One ScalarE
  activation per row-chunk produces W and its row-sum; one VectorE
  tensor_scalar produces the positive-count row-sum.  Partials are summed
  on-chip; the 8 per-core (sum, count) pairs combine on host:
  loss = S / (C + eps).
"""

import numpy as np

import concourse.bass as bass
import concourse.tile as tile
from concourse import bacc, mybir
from concourse.bass_utils import run_bass_kernel_spmd

EPS = 1e-8
BIG = 1e9
B, D, NCORES = 384, 512, 8
PERCORE = B // NCORES  # 48
P = 128
DT = mybir.dt.float32


def _host_prep(labels, clusters, weights):
    """Per-core pair tables (pure label logic, no embedding data)."""
    labels = np.asarray(labels).astype(np.int64)
    clusters = np.asarray(clusters).astype(np.int64)
    weights = np.asarray(weights).astype(np.float32)

    leq = labels[None, :] == labels[:, None]
    rank = np.cumsum(leq.astype(np.int64), axis=1) - 1
    first = leq & (rank % 2 == 1)
    second = leq & (rank % 2 == 0)
    pbase = ~first   # k-mask for in-cluster (i,j) pairs
    qbase = ~second  # k-mask for out-of-cluster pairs

    all_pairs = []
    for c in range(NCORES):
        pairs = []
        for i in range(c * PERCORE, (c + 1) * PERCORE):
            for j in np.where(leq[i])[0]:
                if j != i:
                    pairs.append((i, int(j)))
        all_pairs.append(pairs)
    NP = ((max(len(p) for p in all_pairs) + P - 1) // P) * P

    tables = []
    for c in range(NCORES):
        pairs = all_pairs[c]
        sel = np.zeros((PERCORE, NP), np.float32)
        onej = np.zeros((NP, B), np.float32)
        wvec = np.ones((NP, 1), np.float32)
        madd = np.full((NP, B), -BIG, np.float32)
        for p, (i, j) in enumerate(pairs):
            il = i - c * PERCORE
            sel[il, p] = 1.0
            onej[p, j] = 1.0
            w = float(weights[labels[j]])
            wvec[p, 0] = w
            base = pbase[i] if clusters[i] == clusters[j] else qbase[i]
            mask = base.copy()
            mask[i] = False
            mask[j] = False
            madd[p, :] = np.where(mask, 0.0, np.float32(-BIG * w))
        tables.append(dict(sel=sel, onej=onej, wvec=wvec, madd=madd))
    return tables, NP


def _build_program(NP):
    NCH = NP // P          # pair-row chunks
    NDC = D // P           # contraction chunks (4)

    nc = bacc.Bacc("TRN2", target_bir_lowering=False, debug=False,
                   num_devices=NCORES)

    xt = nc.dram_tensor("xt", [D, B], DT, kind="ExternalInput")
    xmyT2 = nc.dram_tensor("xmyT2", [D, PERCORE], DT, kind="ExternalInput")
    sel = nc.dram_tensor("sel", [PERCORE, NP], DT, kind="ExternalInput")
    madd = nc.dram_tensor("madd", [NP, B], DT, kind="ExternalInput")
    onej = nc.dram_tensor("onej", [NP, B], DT, kind="ExternalInput")
    wvec = nc.dram_tensor("wvec", [NP, 1], DT, kind="ExternalInput")
    out = nc.dram_tensor("out", [1, 2], DT, kind="ExternalOutput")

    with tile.TileContext(nc) as tc:
        with (
            tc.tile_pool(name="cst", bufs=1) as cst,
            tc.tile_pool(name="xtp", bufs=NDC) as xtp,
            tc.tile_pool(name="sq", bufs=2) as sqp,
            tc.tile_pool(name="big", bufs=3) as bigp,
            tc.tile_pool(name="sm", bufs=4) as sm,
            tc.tile_pool(name="ps", bufs=2, space="PSUM") as ps,
            tc.tile_pool(name="vps", bufs=3, space="PSUM") as vps,
        ):
            # --- load inputs ---
            xt_t = []
            for dc in range(NDC):
                t = xtp.tile([P, B], DT, tag="xt")
                nc.sync.dma_start(t[:], xt[dc * P:(dc + 1) * P, :])
                xt_t.append(t)
            xmy_t = []
            for dc in range(NDC):
                t = xtp.tile([P, PERCORE], DT, tag="xmy")
                nc.sync.dma_start(t[:], xmyT2[dc * P:(dc + 1) * P, :])
                xmy_t.append(t)
            sel_t = cst.tile([PERCORE, NP], DT)
            nc.sync.dma_start(sel_t[:], sel[:, :])
            wv_t = cst.tile([P, NCH], DT)
            nc.sync.dma_start(
                wv_t[:], wvec[:, :].rearrange("(c p) o -> p (c o)", p=P))

            negones = cst.tile([P, PERCORE], DT)
            nc.vector.memset(negones[:], -1.0)
            ones1 = cst.tile([P, 1], DT)
            nc.vector.memset(ones1[:], 1.0)

            # --- V[i,k] = 2 x_i.x_k - |x_k|^2  (PE) ---
            v_psum = ps.tile([PERCORE, B], DT)
            for dc in range(NDC):
                nc.tensor.matmul(v_psum[:], lhsT=xmy_t[dc][:], rhs=xt_t[dc][:],
                                 start=(dc == 0), stop=False)
            for dc in range(NDC):
                xsq = sqp.tile([P, B], DT, tag="xsq")
                nc.vector.tensor_mul(xsq[:], xt_t[dc][:], xt_t[dc][:])
                nc.tensor.matmul(v_psum[:], lhsT=negones[:], rhs=xsq[:],
                                 start=False, stop=(dc == NDC - 1))

            v_sb = cst.tile([PERCORE, B], DT)
            nc.scalar.copy(v_sb[:], v_psum[:])

            # --- per-pair-chunk pipeline ---
            sacc = cst.tile([P, NCH], DT)
            cacc = cst.tile([P, NCH], DT)
            for c in range(NCH):
                # Vg[p,k] = V[i_p, k] via one-hot selector matmul
                vg_ps = vps.tile([P, B], DT, tag="vg")
                nc.tensor.matmul(vg_ps[:], lhsT=sel_t[:, c * P:(c + 1) * P],
                                 rhs=v_sb[:], start=True, stop=True)
                # g[p] = V[i_p, j_p] via fused one-hot dot
                oj = bigp.tile([P, B], DT, tag="oj")
                nc.sync.dma_start(oj[:], onej[c * P:(c + 1) * P, :])
                junk = bigp.tile([P, B], DT, tag="junk")
                g = sm.tile([P, 1], DT, tag="g")
                nc.vector.scalar_tensor_tensor(
                    junk[:], in0=vg_ps[:], scalar=1.0, in1=oj[:],
                    op0=mybir.AluOpType.mult, op1=mybir.AluOpType.mult,
                    accum_out=g[:])

                mt = bigp.tile([P, B], DT, tag="mt")
                nc.sync.dma_start(mt[:], madd[c * P:(c + 1) * P, :])

                wcol = wv_t[:, c:c + 1]
                # cvec = w*(1-g)
                cvec = sm.tile([P, 1], DT, tag="cvec")
                nc.vector.tensor_scalar(cvec[:], g[:], -1.0, 1.0,
                                        op0=mybir.AluOpType.mult,
                                        op1=mybir.AluOpType.add)
                nc.vector.tensor_scalar(cvec[:], cvec[:], wcol, None,
                                        op0=mybir.AluOpType.mult)
                # t = w*Vg + madd_w
                t = bigp.tile([P, B], DT, tag="t")
                nc.vector.scalar_tensor_tensor(
                    t[:], in0=vg_ps[:], scalar=wcol, in1=mt[:],
                    op0=mybir.AluOpType.mult, op1=mybir.AluOpType.add)
                # W = relu(t + cvec), row-sum into sacc[:, c]
                w_tile = bigp.tile([P, B], DT, tag="w")
                nc.scalar.activation(w_tile[:], t[:],
                                     mybir.ActivationFunctionType.Relu,
                                     bias=cvec[:, 0:1], scale=1.0,
                                     accum_out=sacc[:, c:c + 1])
                # count = sum(W > EPS) into cacc[:, c]
                cl = bigp.tile([P, B], DT, tag="cl")
                nc.vector.tensor_scalar(cl[:], w_tile[:], EPS, None,
                                        op0=mybir.AluOpType.is_gt,
                                        op1=mybir.AluOpType.add,
                                        accum_out=cacc[:, c:c + 1])

            # --- final reduction ---
            red = cst.tile([P, 2], DT)
            nc.vector.tensor_reduce(red[:, 0:1], sacc[:], mybir.AxisListType.X,
                                    mybir.AluOpType.add)
            nc.vector.tensor_reduce(red[:, 1:2], cacc[:], mybir.AxisListType.X,
                                    mybir.AluOpType.add)
            f_psum = ps.tile([1, 2], DT)
            nc.tensor.matmul(f_psum[:], lhsT=ones1[:], rhs=red[:],
                             start=True, stop=True)
            out_sb = cst.tile([1, 2], DT)
            nc.scalar.copy(out_sb[:], f_psum[:])
            nc.sync.dma_start(out[:, :], out_sb[:])

    nc.compile()
    return nc


def _make_in_maps(embeddings, tables):
    x = np.ascontiguousarray(np.asarray(embeddings, dtype=np.float32))
    xt = np.ascontiguousarray(x.T)  # [D, B]
    in_maps = []
    for c in range(NCORES):
        xmy = x[c * PERCORE:(c + 1) * PERCORE]  # [48, D]
        in_maps.append({
            "xt": xt,
            "xmyT2": np.ascontiguousarray(2.0 * xmy.T),
            "sel": tables[c]["sel"],
            "madd": tables[c]["madd"],
            "onej": tables[c]["onej"],
            "wvec": tables[c]["wvec"],
        })
    return in_maps


def run(embeddings, labels, clusters, weights, trace=False):
    tables, NP = _host_prep(labels, clusters, weights)
    nc = _build_program(NP)
    in_maps = _make_in_maps(embeddings, tables)
    res = run_bass_kernel_spmd(nc, in_maps, core_ids=list(range(NCORES)),
                               trace=trace)
    S = 0.0
    C = 0.0
    for r in res.results:
        S += float(r["out"][0, 0])
        C += float(r["out"][0, 1])
    loss = np.float32(np.float32(S) / np.float32(C + EPS))
    return np.asarray(loss, dtype=np.float32), res


def kernel(embeddings, labels, clusters, weights):
    loss, _ = run(embeddings, labels, clusters, weights)
    return loss
